# revision 2
# baseline (speedup 1.0000x reference)
"""3-layer Elman RNN (tanh) Trainium2 kernel, v2.

Model: x(512,2048) int -> emb(27,20) lookup -> RNN 20->32 -> 32->64 -> 64->64
       -> FC 64->26.  Output (512, 2048, 26) f32.

Sharding: 4-way TIME x 2-way BATCH over 8 cores.  Core (tc, bc) runs
timesteps [128*tc - W, 128*(tc+1)) for batch slice bc*1024:(bc+1)*1024,
with W=24 warmup steps from h=0 (the tanh recurrence contracts ~0.58x
per step, so the splice error is ~1e-6 relative by step W).  Core tc=0
runs W zero-input warmup steps instead; all biases ride the one-hot
path (one-hot columns sum to 1), so zero one-hot keeps its state
exactly 0 and core 0 is exact.

Per core: batch 1024 as 2 ping-pong streams of 512 columns.  Layers are
skewed (step s computes h1(t0+s), h2(t0+s-1), h3(t0+s-2)).  Per
stream-step, 4 wide matmuls (512 moving cols each):
  la   [128x128]@(0,0):  (h2|h3) -> (pre2|pre3),  PSUM bank A  (opens)
  l2co [59x128]@(64,0):  (h1|oh) -> (pre2|pre3) W_ih2 + biases (closes)
  l1co [59x32]@(64,64):  (h1|oh) -> pre1 (W_hh1 + EW'' table w/ b1)
  lf   [64x26]@(64,0):   h3 -> logits (bias added by DVE on PSUM->SBUF)
and 2 ACT ops: tanh(pre2|pre3) [128,512] and tanh(pre1) [32,512].
The one-hot for the NEXT step is DMA'd directly into the state tile
(rows 96:123, cols 512:1024) so (h1|oh) is one contiguous moving
operand; h1 comes out of ACT op2 at rows 64:96, same cols.
A compile-pass removes InstLdweights repeats (adjacent same stationary).
"""

import os
import sys

sys.path.insert(0, "/opt/trn_rl_repo")

import numpy as np

import concourse.bacc as bacc
import concourse.tile as tile
from concourse import inst_simplify
from concourse import mybir
from concourse.tile_rust import add_dep_helper

import ml_dtypes  # noqa: E402

T = 512
B = 2048
NCORES = 8
TC = 4                    # time chunks
BC2 = 2                   # batch shards
N = B // BC2              # batch per core = 1024
SW = N // 2               # stream width = 512
W = 16                    # warmup steps
TK = T // TC              # kept steps per chunk = 128
S = W + TK + 2            # macro steps incl. 2 flush = 154
VOCAB, EMB, H1, H2, H3, OUT = 27, 20, 32, 64, 64, 26

MM_DT = mybir.dt.bfloat16
_NP_OF = {mybir.dt.bfloat16: ml_dtypes.bfloat16, mybir.dt.float32: np.float32}

P1 = 64        # partition where h1 block starts in the state tile
POH = 96       # partition where the one-hot block starts
OC = 512       # col offset of the (h1|oh) region in the state tile
OB = 4         # out-DMA batching (stream-steps per DMA)


class _Bacc(bacc.Bacc):
    """Bacc with an ldweights-dedup pass spliced into compile()."""

    def compile(self):
        self.insert_bir_kernel_barrier_sem_inc()
        self.move_matmul_waits_to_ldweights()
        removed = 0
        pe = mybir.EngineType.PE
        for f in self.m.functions:
            for b in f.blocks:
                insts = b.instructions
                drop = set()
                pend_wait, pend_upd = [], []
                prev_key = None
                for idx, i in enumerate(insts):
                    if getattr(i, "engine", None) != pe:
                        continue  # other engines don't touch the PE array
                    t = type(i).__name__
                    if t == "InstLdweights":
                        key = (str(i.ins[0]), str(i.tile_position),
                               str(i.perf_mode))
                        if key == prev_key:
                            removed += 1
                            drop.add(idx)
                            si = i.sync_info
                            if si is not None:
                                pend_wait.extend(si.on_wait)
                                pend_upd.extend(si.on_update)
                            continue
                        prev_key = key
                    elif t == "InstMatmult":
                        if pend_wait or pend_upd:
                            si = i.sync_info
                            if si is None:
                                si = mybir.SyncInfo(on_wait=[], on_update=[])
                                i.sync_info = si
                            si.on_wait.extend(pend_wait)
                            si.on_update.extend(pend_upd)
                            pend_wait, pend_upd = [], []
                    elif t in ("InstEventSemaphore", "InstNotify",
                               "InstNop"):
                        pass
                    else:
                        prev_key = None
                assert not (pend_wait or pend_upd)
                b.instructions = [x for k, x in enumerate(insts)
                                  if k not in drop]
        self.generate_event_semaphores()
        self.remove_dead_instructions_after_branch()
        self.validate_blocks()
        self.dce_regs()
        self.thread_jumps()
        self.remove_dead_blocks()
        self.remove_dead_allocations()
        self.verify_switch_hints()
        self.alloc_regs()
        inst_simplify.simplify(self)
        self.fuse_regops()
        self.fuse_blocks()
        self.replace_nops_with_events()
        for engine in self.engines:
            self.fuse_nops(engine)
        self.remove_dead_nops()
        self.remove_dangling_data()
        self.generate_event_semaphores()
        self.insert_library_loads()
        self.insert_act_table_loads()
        self.insert_hostgen_rebases()
        self.codegen_inst_isa_subclasses()


def _build_nc():
    nc = _Bacc()
    f32 = mybir.dt.float32
    mdt = MM_DT

    oh_d = nc.dram_tensor("oh", [VOCAB, S * N], mdt, kind="ExternalInput")
    la_d = nc.dram_tensor("la", [H2 + H3, H2 + H3], mdt, kind="ExternalInput")
    l2_d = nc.dram_tensor("l2", [H1, H2], mdt, kind="ExternalInput")
    l1_d = nc.dram_tensor("l1", [H1 + VOCAB, H1], mdt, kind="ExternalInput")
    lf_d = nc.dram_tensor("lf", [H3, OUT], mdt, kind="ExternalInput")
    bf_d = nc.dram_tensor("bf", [OUT, 1], f32, kind="ExternalInput")
    # b23i: per-core INITIAL bias (zero for tc=0 cores so their zero-input
    # warmup keeps state exactly 0); b23r: the real bias, DMA'd over the
    # live tile at s=W+1 (b2 rows) / s=W+2 (b3 rows) on every core.
    b23i_d = nc.dram_tensor("b23i", [128, 1], f32, kind="ExternalInput")
    b23r_d = nc.dram_tensor("b23r", [128, 1], f32, kind="ExternalInput")
    o_d = nc.dram_tensor("o", [OUT, TK * N], f32, kind="ExternalOutput")

    tanh = mybir.ActivationFunctionType.Tanh

    with tile.TileContext(nc) as tc_:
        with (
            tc_.tile_pool(name="wpool", bufs=1) as wpool,
            tc_.tile_pool(name="hpool", bufs=6) as hpool,
            tc_.tile_pool(name="opool", bufs=3) as opool,
            tc_.tile_pool(name="ppool", bufs=1, space="PSUM") as ppool,
            tc_.tile_pool(name="fcpool", bufs=2, space="PSUM") as fcpool,
            tc_.tile_pool(name="warmp", bufs=1, space="PSUM") as warmp,
        ):
            la = wpool.tile([H2 + H3, H2 + H3], mdt)
            wih2 = wpool.tile([P1 + H1, H2], mdt)           # rows 64:96
            l1co = wpool.tile([P1 + H1 + VOCAB, H1], mdt)   # rows 64:123
            lf = wpool.tile([P1 + H3, OUT], mdt)            # rows 64:128
            bfc = wpool.tile([OUT, 1], f32)
            b23 = wpool.tile([128, 1], f32)
            nc.sync.dma_start(la[:], la_d[:])
            nc.sync.dma_start(wih2[P1:P1 + H1, :], l2_d[:])
            nc.sync.dma_start(l1co[P1:P1 + H1 + VOCAB, :], l1_d[:])
            nc.sync.dma_start(lf[P1:P1 + H3, :], lf_d[:])
            nc.sync.dma_start(bfc[:], bf_d[:])
            nc.sync.dma_start(b23[:], b23i_d[:])

            # Persistent PSUM banks: per stream, bank A (pre2|pre3) and
            # bank B (pre1 on rows 64:96).  WAR deps via tile tracking.
            psA = [ppool.tile([128, SW], f32, name=f"psA{i}")
                   for i in range(2)]
            psB = [ppool.tile([128, SW], f32, name=f"psB{i}")
                   for i in range(2)]

            # Initial state tiles: zeros + one-hot for token 0.
            hs = []
            for stream in range(2):
                h0 = hpool.tile([128, 2 * SW], mdt)
                nc.vector.memset(h0[:], 0.0)
                nc.sync.dma_start(
                    h0[POH:POH + VOCAB, OC:OC + SW],
                    oh_d[:, stream * SW:stream * SW + SW])
                hs.append(h0)

            # PE warmup: back-to-back wide matmuls trip the HAM clock gate
            # to 8/8; the first real matmul is dep-chained onto the last
            # warmup mm so the PE has no idle window at loop start (an
            # idle MID window would re-throttle to 4/8 for the whole run).
            warm = wpool.tile([128, 512], mdt)
            nc.vector.memset(warm[:], 0.0)
            wp = warmp.tile([128, 512], f32)
            last_warm = None
            for _ in range(20):
                last_warm = nc.tensor.matmul(wp[:], warm[:, 0:128], warm[:],
                                             start=True, stop=True)

            outbuf = [None, None]

            for s in range(S):
                if s == W + 1:
                    nc.sync.dma_start(b23[0:H2, :], b23r_d[0:H2, :])
                elif s == W + 2:
                    nc.sync.dma_start(b23[H2:128, :], b23r_d[H2:128, :])
                hnew = [None, None]
                for stream in range(2):
                    hp = hs[stream]
                    pa, pb = psA[stream], psB[stream]
                    mm_la = nc.tensor.matmul(pa[:, :], la[:], hp[:, 0:SW],
                                             start=True, stop=False)
                    if s == 0 and stream == 0:
                        add_dep_helper(mm_la.ins, last_warm.ins, sync=False,
                                       reason="no PE idle between warmup "
                                              "burst and loop (HAM)")
                    nc.tensor.matmul(pa[0:H2, :],
                                     wih2[P1:P1 + H1, :],
                                     hp[P1:P1 + H1, OC:OC + SW],
                                     start=False, stop=True,
                                     tile_position=(P1, 0),
                                     skip_group_check=True)
                    nc.tensor.matmul(pb[P1:P1 + H1, :],
                                     l1co[P1:P1 + H1 + VOCAB, :],
                                     hp[P1:P1 + H1 + VOCAB, OC:OC + SW],
                                     start=True, stop=True,
                                     tile_position=(P1, P1))
                    # HAM feeders: dependency-free wide matmuls keep the PE
                    # streaming through what would be an idle wait (any
                    # micro-idle re-throttles the clock gate to 4/8).
                    nc.tensor.matmul(wp[:], warm[:, 0:128], warm[:],
                                     start=True, stop=True)
                    nc.tensor.matmul(wp[:], warm[:, 0:128], warm[:],
                                     start=True, stop=True)
                    hn = hpool.tile([128, 2 * SW], mdt)
                    # one-hot for step s+1 lands in hn (read by s+1's mms)
                    tok = s + 1
                    if tok < S:
                        nc.sync.dma_start(
                            hn[POH:POH + VOCAB, OC:OC + SW],
                            oh_d[:, tok * N + stream * SW:
                                 tok * N + stream * SW + SW])
                    nc.scalar.activation(hn[:, 0:SW], pa[:, :], tanh,
                                         bias=b23[:])
                    nc.scalar.activation(hn[P1:P1 + H1, OC:OC + SW],
                                         pb[P1:P1 + H1, :], tanh)
                    if s == 0:
                        nc.vector.memset(hn[:, 0:SW], 0.0)
                    elif s == 1:
                        nc.vector.memset(hn[H2:128, 0:SW], 0.0)
                    hnew[stream] = hn
                hs = hnew
                # FC for kept steps, both streams adjacent (lf LDW dedups)
                if W + 2 <= s < W + 2 + TK:
                    k = s - (W + 2)
                    j = k % OB
                    for stream in range(2):
                        fco = fcpool.tile([OUT, SW], f32)
                        nc.tensor.matmul(fco[:, :], lf[P1:P1 + H3, :],
                                         hs[stream][P1:128, 0:SW],
                                         start=True, stop=True,
                                         tile_position=(P1, 0))
                        if j == 0 and stream == 0:
                            outbuf[0] = opool.tile([OUT, OB * N], f32,
                                                   name="ob")
                        ob = outbuf[0]
                        nc.vector.tensor_scalar_add(
                            ob[:, j * N + stream * SW:
                               j * N + stream * SW + SW],
                            fco[:, :], bfc[:])
                        if j == OB - 1 and stream == 1:
                            c0 = (k - j) * N
                            nc.sync.dma_start(
                                o_d[:, c0:c0 + OB * N], ob[:])
    nc.compile()
    return nc


_NC_CACHE = None


def _get_nc():
    global _NC_CACHE
    if _NC_CACHE is None:
        _NC_CACHE = _build_nc()
    return _NC_CACHE


def _prep_inputs(inputs):
    npdt = _NP_OF[MM_DT]
    f32 = np.float32
    x = np.asarray(inputs["x"]).astype(np.int64)             # (T, B)
    emb = np.asarray(inputs["emb"], f32)
    W_ih1 = np.asarray(inputs["W_ih1"], f32)
    W_hh1 = np.asarray(inputs["W_hh1"], f32)
    b1 = np.asarray(inputs["b_ih1"], f32) + np.asarray(inputs["b_hh1"], f32)
    W_ih2 = np.asarray(inputs["W_ih2"], f32)
    W_hh2 = np.asarray(inputs["W_hh2"], f32)
    b2 = np.asarray(inputs["b_ih2"], f32) + np.asarray(inputs["b_hh2"], f32)
    W_ih3 = np.asarray(inputs["W_ih3"], f32)
    W_hh3 = np.asarray(inputs["W_hh3"], f32)
    b3 = np.asarray(inputs["b_ih3"], f32) + np.asarray(inputs["b_hh3"], f32)
    W_fc = np.asarray(inputs["W_fc"], f32)
    b_fc = np.asarray(inputs["b_fc"], f32)

    la = np.zeros((H2 + H3, H2 + H3), f32)
    la[0:H2, 0:H2] = W_hh2.T
    la[0:H2, H2:] = W_ih3.T
    la[H2:, H2:] = W_hh3.T
    l2 = W_ih2.T.copy()                                      # [32, 64]
    # l1co rows 0:32 = h1 -> W_hh1^T; rows 32:59 = one-hot -> EW'' table
    # (embedding @ W_ih1^T + b1).
    l1 = np.zeros((H1 + VOCAB, H1), f32)
    l1[0:H1, :] = W_hh1.T
    l1[H1:, :] = emb @ W_ih1.T + b1[None, :]
    lf = W_fc.T.copy()                                       # [64, 26]
    bf = b_fc.reshape(OUT, 1).astype(f32)
    b23r = np.concatenate([b2, b3]).reshape(128, 1).astype(f32)

    shared = {
        "la": la.astype(npdt), "l2": l2.astype(npdt),
        "l1": l1.astype(npdt), "lf": lf.astype(npdt), "bf": bf,
        "b23r": b23r,
    }
    eye = np.eye(VOCAB, dtype=npdt)
    in_maps = []
    for core in range(NCORES):
        tcid, bcid = divmod(core, BC2)
        t0 = TK * tcid - W
        xc = x[:, bcid * N:(bcid + 1) * N]                   # (T, N)
        oh = np.zeros((VOCAB, S * N), npdt)
        for k in range(S - 2):
            t = t0 + k
            if 0 <= t < T:
                oh[:, k * N:(k + 1) * N] = eye[xc[t]].T
        b23i = np.zeros((128, 1), f32) if tcid == 0 else b23r
        in_maps.append(dict(shared, oh=np.ascontiguousarray(oh), b23i=b23i))
    return in_maps


def _assemble(results):
    out = np.empty((T, B, OUT), np.float32)
    for core in range(NCORES):
        tcid, bcid = divmod(core, BC2)
        o = results[core]["o"].reshape(OUT, TK, N).transpose(1, 2, 0)
        out[TK * tcid:TK * (tcid + 1), bcid * N:(bcid + 1) * N, :] = o
    return out


def _run(inputs, **spmd_kwargs):
    from concourse.bass_utils import run_bass_kernel_spmd
    nc = _get_nc()
    in_maps = _prep_inputs(inputs)
    res = run_bass_kernel_spmd(nc, in_maps, core_ids=list(range(NCORES)),
                               **spmd_kwargs)
    return _assemble(res.results), res


def kernel(**inputs) -> np.ndarray:
    return _run(inputs)[0]


if __name__ == "__main__":
    import reference as R
    ins = {k: np.asarray(v) for k, v in R.setup_inputs().items()}
    got = kernel(**ins)
    import jax.numpy as jnp
    want = np.asarray(R.reference(**{k: jnp.asarray(v) for k, v in ins.items()}))
    err = np.abs(got - want)
    print("absmax", err.max(), "rel", err.max() / np.abs(want).max())


# revision 3
# speedup vs baseline: 1.0039x; 1.0039x over previous
"""3-layer Elman RNN (tanh) Trainium2 kernel, v2.

Model: x(512,2048) int -> emb(27,20) lookup -> RNN 20->32 -> 32->64 -> 64->64
       -> FC 64->26.  Output (512, 2048, 26) f32.

Sharding: 4-way TIME x 2-way BATCH over 8 cores.  Core (tc, bc) runs
timesteps [128*tc - W, 128*(tc+1)) for batch slice bc*1024:(bc+1)*1024,
with W=24 warmup steps from h=0 (the tanh recurrence contracts ~0.58x
per step, so the splice error is ~1e-6 relative by step W).  Core tc=0
runs W zero-input warmup steps instead; all biases ride the one-hot
path (one-hot columns sum to 1), so zero one-hot keeps its state
exactly 0 and core 0 is exact.

Per core: batch 1024 as 2 ping-pong streams of 512 columns.  Layers are
skewed (step s computes h1(t0+s), h2(t0+s-1), h3(t0+s-2)).  Per
stream-step, 4 wide matmuls (512 moving cols each):
  la   [128x128]@(0,0):  (h2|h3) -> (pre2|pre3),  PSUM bank A  (opens)
  l2co [59x128]@(64,0):  (h1|oh) -> (pre2|pre3) W_ih2 + biases (closes)
  l1co [59x32]@(64,64):  (h1|oh) -> pre1 (W_hh1 + EW'' table w/ b1)
  lf   [64x26]@(64,0):   h3 -> logits (bias added by DVE on PSUM->SBUF)
and 2 ACT ops: tanh(pre2|pre3) [128,512] and tanh(pre1) [32,512].
The one-hot for the NEXT step is DMA'd directly into the state tile
(rows 96:123, cols 512:1024) so (h1|oh) is one contiguous moving
operand; h1 comes out of ACT op2 at rows 64:96, same cols.
A compile-pass removes InstLdweights repeats (adjacent same stationary).
"""

import os
import sys

sys.path.insert(0, "/opt/trn_rl_repo")

import numpy as np

import concourse.bacc as bacc
import concourse.tile as tile
from concourse import inst_simplify
from concourse import mybir
from concourse.tile_rust import add_dep_helper

import ml_dtypes  # noqa: E402

T = 512
B = 2048
NCORES = 8
TC = 4                    # time chunks
BC2 = 2                   # batch shards
N = B // BC2              # batch per core = 1024
SW = N // 2               # stream width = 512
W = 16                    # warmup steps
TK = T // TC              # kept steps per chunk = 128
S = W + TK + 2            # macro steps incl. 2 flush = 154
VOCAB, EMB, H1, H2, H3, OUT = 27, 20, 32, 64, 64, 26

MM_DT = mybir.dt.bfloat16
_NP_OF = {mybir.dt.bfloat16: ml_dtypes.bfloat16, mybir.dt.float32: np.float32}

P1 = 64        # partition where h1 block starts in the state tile
POH = 96       # partition where the one-hot block starts
OC = 512       # col offset of the (h1|oh) region in the state tile
OB = 4         # out-DMA batching (stream-steps per DMA)


class _Bacc(bacc.Bacc):
    """Bacc with an ldweights-dedup pass spliced into compile()."""

    def compile(self):
        self.insert_bir_kernel_barrier_sem_inc()
        self.move_matmul_waits_to_ldweights()
        removed = 0
        pe = mybir.EngineType.PE
        for f in self.m.functions:
            for b in f.blocks:
                insts = b.instructions
                drop = set()
                pend_wait, pend_upd = [], []
                prev_key = None
                for idx, i in enumerate(insts):
                    if getattr(i, "engine", None) != pe:
                        continue  # other engines don't touch the PE array
                    t = type(i).__name__
                    if t == "InstLdweights":
                        key = (str(i.ins[0]), str(i.tile_position),
                               str(i.perf_mode))
                        if key == prev_key:
                            removed += 1
                            drop.add(idx)
                            si = i.sync_info
                            if si is not None:
                                pend_wait.extend(si.on_wait)
                                pend_upd.extend(si.on_update)
                            continue
                        prev_key = key
                    elif t == "InstMatmult":
                        if pend_wait or pend_upd:
                            si = i.sync_info
                            if si is None:
                                si = mybir.SyncInfo(on_wait=[], on_update=[])
                                i.sync_info = si
                            si.on_wait.extend(pend_wait)
                            si.on_update.extend(pend_upd)
                            pend_wait, pend_upd = [], []
                    elif t in ("InstEventSemaphore", "InstNotify",
                               "InstNop"):
                        pass
                    else:
                        prev_key = None
                assert not (pend_wait or pend_upd)
                b.instructions = [x for k, x in enumerate(insts)
                                  if k not in drop]
        self.generate_event_semaphores()
        self.remove_dead_instructions_after_branch()
        self.validate_blocks()
        self.dce_regs()
        self.thread_jumps()
        self.remove_dead_blocks()
        self.remove_dead_allocations()
        self.verify_switch_hints()
        self.alloc_regs()
        inst_simplify.simplify(self)
        self.fuse_regops()
        self.fuse_blocks()
        self.replace_nops_with_events()
        for engine in self.engines:
            self.fuse_nops(engine)
        self.remove_dead_nops()
        self.remove_dangling_data()
        self.generate_event_semaphores()
        self.insert_library_loads()
        self.insert_act_table_loads()
        self.insert_hostgen_rebases()
        self.codegen_inst_isa_subclasses()


def _build_nc():
    nc = _Bacc()
    f32 = mybir.dt.float32
    mdt = MM_DT

    oh_d = nc.dram_tensor("oh", [VOCAB, S * N], mdt, kind="ExternalInput")
    la_d = nc.dram_tensor("la", [H2 + H3, H2 + H3], mdt, kind="ExternalInput")
    l2_d = nc.dram_tensor("l2", [H1, H2], mdt, kind="ExternalInput")
    l1_d = nc.dram_tensor("l1", [H1 + VOCAB, H1], mdt, kind="ExternalInput")
    lf_d = nc.dram_tensor("lf", [H3, OUT], mdt, kind="ExternalInput")
    bf_d = nc.dram_tensor("bf", [OUT, 1], f32, kind="ExternalInput")
    # b23i: per-core INITIAL bias (zero for tc=0 cores so their zero-input
    # warmup keeps state exactly 0); b23r: the real bias, DMA'd over the
    # live tile at s=W+1 (b2 rows) / s=W+2 (b3 rows) on every core.
    b23i_d = nc.dram_tensor("b23i", [128, 1], f32, kind="ExternalInput")
    b23r_d = nc.dram_tensor("b23r", [128, 1], f32, kind="ExternalInput")
    o_d = nc.dram_tensor("o", [OUT, TK * N], f32, kind="ExternalOutput")

    tanh = mybir.ActivationFunctionType.Tanh

    with tile.TileContext(nc) as tc_:
        with (
            tc_.tile_pool(name="wpool", bufs=1) as wpool,
            tc_.tile_pool(name="hpool", bufs=6) as hpool,
            tc_.tile_pool(name="opool", bufs=3) as opool,
            tc_.tile_pool(name="ppool", bufs=1, space="PSUM") as ppool,
            tc_.tile_pool(name="fcpool", bufs=2, space="PSUM") as fcpool,
            tc_.tile_pool(name="warmp", bufs=1, space="PSUM") as warmp,
        ):
            la = wpool.tile([H2 + H3, H2 + H3], mdt)
            wih2 = wpool.tile([P1 + H1, H2], mdt)           # rows 64:96
            l1co = wpool.tile([P1 + H1 + VOCAB, H1], mdt)   # rows 64:123
            lf = wpool.tile([P1 + H3, OUT], mdt)            # rows 64:128
            bfc = wpool.tile([OUT, 1], f32)
            b23 = wpool.tile([128, 1], f32)
            nc.sync.dma_start(la[:], la_d[:])
            nc.sync.dma_start(wih2[P1:P1 + H1, :], l2_d[:])
            nc.sync.dma_start(l1co[P1:P1 + H1 + VOCAB, :], l1_d[:])
            nc.sync.dma_start(lf[P1:P1 + H3, :], lf_d[:])
            nc.sync.dma_start(bfc[:], bf_d[:])
            nc.sync.dma_start(b23[:], b23i_d[:])

            # Persistent PSUM banks: per stream, bank A (pre2|pre3) and
            # bank B (pre1 on rows 64:96).  WAR deps via tile tracking.
            psA = [ppool.tile([128, SW], f32, name=f"psA{i}")
                   for i in range(2)]
            psB = [ppool.tile([128, SW], f32, name=f"psB{i}")
                   for i in range(2)]

            # Initial state tiles: zeros + one-hot for token 0.
            hs = []
            for stream in range(2):
                h0 = hpool.tile([128, 2 * SW], mdt)
                nc.vector.memset(h0[:], 0.0)
                nc.sync.dma_start(
                    h0[POH:POH + VOCAB, OC:OC + SW],
                    oh_d[:, stream * SW:stream * SW + SW])
                hs.append(h0)

            # PE warmup: back-to-back wide matmuls trip the HAM clock gate
            # to 8/8; the first real matmul is dep-chained onto the last
            # warmup mm so the PE has no idle window at loop start (an
            # idle MID window would re-throttle to 4/8 for the whole run).
            warm = wpool.tile([128, 512], mdt)
            nc.vector.memset(warm[:], 0.0)
            wp = warmp.tile([128, 512], f32)
            last_warm = None
            for _ in range(20):
                last_warm = nc.tensor.matmul(wp[:], warm[:, 0:128], warm[:],
                                             start=True, stop=True)

            outbuf = [None, None]

            for s in range(S):
                if s == W + 1:
                    nc.sync.dma_start(b23[0:H2, :], b23r_d[0:H2, :])
                elif s == W + 2:
                    nc.sync.dma_start(b23[H2:128, :], b23r_d[H2:128, :])
                hnew = [None, None]
                for stream in range(2):
                    hp = hs[stream]
                    pa, pb = psA[stream], psB[stream]
                    mm_la = nc.tensor.matmul(pa[:, :], la[:], hp[:, 0:SW],
                                             start=True, stop=False)
                    if s == 0 and stream == 0:
                        add_dep_helper(mm_la.ins, last_warm.ins, sync=False,
                                       reason="no PE idle between warmup "
                                              "burst and loop (HAM)")
                    nc.tensor.matmul(pa[0:H2, :],
                                     wih2[P1:P1 + H1, :],
                                     hp[P1:P1 + H1, OC:OC + SW],
                                     start=False, stop=True,
                                     tile_position=(P1, 0),
                                     skip_group_check=True)
                    nc.tensor.matmul(pb[P1:P1 + H1, :],
                                     l1co[P1:P1 + H1 + VOCAB, :],
                                     hp[P1:P1 + H1 + VOCAB, OC:OC + SW],
                                     start=True, stop=True,
                                     tile_position=(P1, P1))
                    # HAM feeders: dependency-free wide matmuls keep the PE
                    # streaming through what would be an idle wait (any
                    # micro-idle re-throttles the clock gate to 4/8).
                    nc.tensor.matmul(wp[:], warm[:, 0:128], warm[:],
                                     start=True, stop=True)
                    if stream == 0:
                        nc.tensor.matmul(wp[:], warm[:, 0:128], warm[:],
                                         start=True, stop=True)
                    hn = hpool.tile([128, 2 * SW], mdt)
                    # one-hot for step s+1 lands in hn (read by s+1's mms)
                    tok = s + 1
                    if tok < S:
                        nc.sync.dma_start(
                            hn[POH:POH + VOCAB, OC:OC + SW],
                            oh_d[:, tok * N + stream * SW:
                                 tok * N + stream * SW + SW])
                    nc.scalar.activation(hn[:, 0:SW], pa[:, :], tanh,
                                         bias=b23[:])
                    nc.scalar.activation(hn[P1:P1 + H1, OC:OC + SW],
                                         pb[P1:P1 + H1, :], tanh)
                    if s == 0:
                        nc.vector.memset(hn[:, 0:SW], 0.0)
                    elif s == 1:
                        nc.vector.memset(hn[H2:128, 0:SW], 0.0)
                    hnew[stream] = hn
                hs = hnew
                # FC for kept steps, both streams adjacent (lf LDW dedups)
                if W + 2 <= s < W + 2 + TK:
                    k = s - (W + 2)
                    j = k % OB
                    for stream in range(2):
                        fco = fcpool.tile([OUT, SW], f32)
                        nc.tensor.matmul(fco[:, :], lf[P1:P1 + H3, :],
                                         hs[stream][P1:128, 0:SW],
                                         start=True, stop=True,
                                         tile_position=(P1, 0))
                        if j == 0 and stream == 0:
                            outbuf[0] = opool.tile([OUT, OB * N], f32,
                                                   name="ob")
                        ob = outbuf[0]
                        nc.vector.tensor_scalar_add(
                            ob[:, j * N + stream * SW:
                               j * N + stream * SW + SW],
                            fco[:, :], bfc[:])
                        if j == OB - 1 and stream == 1:
                            c0 = (k - j) * N
                            nc.sync.dma_start(
                                o_d[:, c0:c0 + OB * N], ob[:])
    nc.compile()
    return nc


_NC_CACHE = None


def _get_nc():
    global _NC_CACHE
    if _NC_CACHE is None:
        _NC_CACHE = _build_nc()
    return _NC_CACHE


def _prep_inputs(inputs):
    npdt = _NP_OF[MM_DT]
    f32 = np.float32
    x = np.asarray(inputs["x"]).astype(np.int64)             # (T, B)
    emb = np.asarray(inputs["emb"], f32)
    W_ih1 = np.asarray(inputs["W_ih1"], f32)
    W_hh1 = np.asarray(inputs["W_hh1"], f32)
    b1 = np.asarray(inputs["b_ih1"], f32) + np.asarray(inputs["b_hh1"], f32)
    W_ih2 = np.asarray(inputs["W_ih2"], f32)
    W_hh2 = np.asarray(inputs["W_hh2"], f32)
    b2 = np.asarray(inputs["b_ih2"], f32) + np.asarray(inputs["b_hh2"], f32)
    W_ih3 = np.asarray(inputs["W_ih3"], f32)
    W_hh3 = np.asarray(inputs["W_hh3"], f32)
    b3 = np.asarray(inputs["b_ih3"], f32) + np.asarray(inputs["b_hh3"], f32)
    W_fc = np.asarray(inputs["W_fc"], f32)
    b_fc = np.asarray(inputs["b_fc"], f32)

    la = np.zeros((H2 + H3, H2 + H3), f32)
    la[0:H2, 0:H2] = W_hh2.T
    la[0:H2, H2:] = W_ih3.T
    la[H2:, H2:] = W_hh3.T
    l2 = W_ih2.T.copy()                                      # [32, 64]
    # l1co rows 0:32 = h1 -> W_hh1^T; rows 32:59 = one-hot -> EW'' table
    # (embedding @ W_ih1^T + b1).
    l1 = np.zeros((H1 + VOCAB, H1), f32)
    l1[0:H1, :] = W_hh1.T
    l1[H1:, :] = emb @ W_ih1.T + b1[None, :]
    lf = W_fc.T.copy()                                       # [64, 26]
    bf = b_fc.reshape(OUT, 1).astype(f32)
    b23r = np.concatenate([b2, b3]).reshape(128, 1).astype(f32)

    shared = {
        "la": la.astype(npdt), "l2": l2.astype(npdt),
        "l1": l1.astype(npdt), "lf": lf.astype(npdt), "bf": bf,
        "b23r": b23r,
    }
    eye = np.eye(VOCAB, dtype=npdt)
    in_maps = []
    for core in range(NCORES):
        tcid, bcid = divmod(core, BC2)
        t0 = TK * tcid - W
        xc = x[:, bcid * N:(bcid + 1) * N]                   # (T, N)
        oh = np.zeros((VOCAB, S * N), npdt)
        for k in range(S - 2):
            t = t0 + k
            if 0 <= t < T:
                oh[:, k * N:(k + 1) * N] = eye[xc[t]].T
        b23i = np.zeros((128, 1), f32) if tcid == 0 else b23r
        in_maps.append(dict(shared, oh=np.ascontiguousarray(oh), b23i=b23i))
    return in_maps


def _assemble(results):
    out = np.empty((T, B, OUT), np.float32)
    for core in range(NCORES):
        tcid, bcid = divmod(core, BC2)
        o = results[core]["o"].reshape(OUT, TK, N).transpose(1, 2, 0)
        out[TK * tcid:TK * (tcid + 1), bcid * N:(bcid + 1) * N, :] = o
    return out


def _run(inputs, **spmd_kwargs):
    from concourse.bass_utils import run_bass_kernel_spmd
    nc = _get_nc()
    in_maps = _prep_inputs(inputs)
    res = run_bass_kernel_spmd(nc, in_maps, core_ids=list(range(NCORES)),
                               **spmd_kwargs)
    return _assemble(res.results), res


def kernel(**inputs) -> np.ndarray:
    return _run(inputs)[0]


if __name__ == "__main__":
    import reference as R
    ins = {k: np.asarray(v) for k, v in R.setup_inputs().items()}
    got = kernel(**ins)
    import jax.numpy as jnp
    want = np.asarray(R.reference(**{k: jnp.asarray(v) for k, v in ins.items()}))
    err = np.abs(got - want)
    print("absmax", err.max(), "rel", err.max() / np.abs(want).max())
